# revision 1
# baseline (speedup 1.0000x reference)
"""Multi-head self-attention Trainium2 kernel (8-core SPMD, full IO).

Problem: x:(2,2048,1024) f32; Wq/Wk/Wv/Wo:(1024,1024); bo:(1024,)
  out = softmax((xWq)(xWk)^T / 8) (xWv) reshaped @ Wo + bo

Sharding: data parallel on batch N=2 x tensor parallel on 16 heads in
4 groups of 4 heads.  Core c handles batch c//4, heads [4*(c%4), 4*(c%4)+4).
Each core computes a partial fc_out product (2048,1024); the host sums the
4 head-group partials per batch and adds the bias.

On-chip layout (per core):
  xT   (1024,2048)  x[n]^T, embed on partitions (8 chunks of 128)
  Q^T/K^T stored as [128, 2, 2048] (dims-chunk on partitions, tokens free)
  V    stored as [128(tokens), 16, 4, 65]; col 64 = ones (denominator trick)
  scores are computed TRANSPOSED: S^T[k,q] so that exp runs on ACT and the
  softmax denominator falls out of the ones-column of V during the O^T
  accumulation (row 64 of the [65,512] psum).  No max subtraction: scores
  are ~N(0,1), bounded well inside fp32 exp range (as in the reference,
  which subtracts max only for stability, not value).
"""

import os

import numpy as np

import concourse.bass as bass
import concourse.tile as tile
from concourse import bacc, mybir
from concourse import bass_utils

F32 = mybir.dt.float32

EMBED = 1024
SEQ = 2048
NB = 2  # batch
HEADS = 16
D = 64  # head dim
NCORES = 8
GROUPS = 4  # head groups (tensor parallel)
HG = HEADS // GROUPS  # heads per core = 4
DG = HG * D  # dims per core = 256

# matmul operand dtype:
#   float32  - exact, 1/4 PE rate
#   float32r - tf32-class (~3e-4 rel), ~2 cyc/row (4-byte stream bound)
#   bfloat16 - ~5e-3 rel, full PE rate, half DMA/SBUF footprint
_MM_DTYPE_NAME = os.environ.get("MHA_MM_DTYPE", "bfloat16")
MM_DTYPE = getattr(mybir.dt, _MM_DTYPE_NAME)
BF16 = mybir.dt.bfloat16

# set by run_cores(); test.py reads exec_time_ns from here
LAST_RESULTS = None
_CACHED_NC = {}


MD = MM_DTYPE  # dtype of matmul-feeding tiles
# DRAM dtype of the big inputs: bf16 inputs are converted host-side (DMA
# cannot cast); f32r shares fp32 bits so DRAM stays f32 + bitcast at DMA.
IN_DT = BF16 if MM_DTYPE == BF16 else F32
IN_NP = None  # numpy dtype for host conversion, set below


def _in_cast(ap):
    """DRAM-side view of an input AP in the matmul dtype."""
    return ap if MD in (F32, BF16) else ap.bitcast(MD)


def build_nc():
    nc = bacc.Bacc("TRN2", target_bir_lowering=False, debug=False,
                   num_devices=NCORES)

    xT = nc.dram_tensor("xT", (EMBED, SEQ), IN_DT, kind="ExternalInput").ap()
    wq = nc.dram_tensor("wq", (EMBED, DG), IN_DT, kind="ExternalInput").ap()
    wk = nc.dram_tensor("wk", (EMBED, DG), IN_DT, kind="ExternalInput").ap()
    wv = nc.dram_tensor("wv", (EMBED, DG), IN_DT, kind="ExternalInput").ap()
    wo = nc.dram_tensor("wo", (DG, EMBED), IN_DT, kind="ExternalInput").ap()
    y = nc.dram_tensor("y", (SEQ, EMBED), F32, kind="ExternalOutput").ap()
    # DRAM bounce buffers for the softmax denominators: SBUF sources can't be
    # partition-broadcast by DMA, DRAM sources can.
    den_dram = nc.dram_tensor("den_scratch", (HG, SEQ), F32).ap()
    rden_dram = nc.dram_tensor("rden_scratch", (HG, SEQ), F32).ap()

    KC = EMBED // 128  # 8 contraction chunks for projections

    with tile.TileContext(nc) as tc:
        with (
            tc.tile_pool(name="weights", bufs=1) as wpool,
            tc.tile_pool(name="qk", bufs=1) as qkpool,
            tc.tile_pool(name="vpool", bufs=1) as vpool,
            tc.tile_pool(name="otpool", bufs=1) as otpool,
            tc.tile_pool(name="xchunk", bufs=2) as xpool,
            tc.tile_pool(name="epool", bufs=12) as epool,
            tc.tile_pool(name="stage", bufs=4) as stpool,
            tc.tile_pool(name="den", bufs=1) as denpool,
            tc.tile_pool(name="rbc", bufs=2) as rbcpool,
            tc.tile_pool(name="ystage", bufs=3) as ypool,
            tc.tile_pool(name="psum", bufs=2, space="PSUM") as pspool,
            tc.tile_pool(name="psum_o", bufs=2, space="PSUM") as popool,
        ):
            # ---- load weights ----
            wq_sb = wpool.tile([128, KC, DG], MD)
            wk_sb = wpool.tile([128, KC, DG], MD)
            wv_sb = wpool.tile([128, KC, DG], MD)
            wo_sb = wpool.tile([128, DG // 128, EMBED], MD)
            nc.sync.dma_start(out=wq_sb, in_=_in_cast(wq).rearrange("(c p) n -> p c n", p=128))
            nc.sync.dma_start(out=wk_sb, in_=_in_cast(wk).rearrange("(c p) n -> p c n", p=128))
            nc.sync.dma_start(out=wv_sb, in_=_in_cast(wv).rearrange("(c p) n -> p c n", p=128))
            nc.sync.dma_start(out=wo_sb, in_=_in_cast(wo).rearrange("(c p) n -> p c n", p=128))

            # per-512-token-chunk tiles: finer dependency granularity lets
            # phase-2 attention start as soon as its chunks are projected
            QTs = [qkpool.tile([128, 2, 512], MD, name=f"qt{t}", tag=f"qt{t}")
                   for t in range(4)]
            KTs = [qkpool.tile([128, 2, 512], MD, name=f"kt{t}", tag=f"kt{t}")
                   for t in range(4)]
            Vs = [vpool.tile([128, 4, HG, D + 1], MD, name=f"v{t}", tag=f"v{t}")
                  for t in range(4)]
            for t in range(4):
                ones_col = Vs[t][:, :, :, D:D + 1]
                nc.vector.memset(
                    ones_col.bitcast(F32) if MD == mybir.dt.float32r
                    else ones_col, 1.0)

            xTr = _in_cast(xT).rearrange("(c p) s -> p c s", p=128)

            # ---- phase 1: projections, one 256-token chunk at a time ----
            TCH = 512
            for tcb in range(SEQ // TCH):
                xc = xpool.tile([128, KC, TCH], MD)
                nc.sync.dma_start(out=xc, in_=xTr[:, :, tcb * TCH:(tcb + 1) * TCH])

                # Q^T and K^T chunks: [dims 128, tokens TCH]
                for wsb, dst in ((wq_sb, QTs), (wk_sb, KTs)):
                    for mt in range(2):
                        ps = popool.tile([128, 512], F32, name="ps1", tag="po")
                        for kc in range(KC):
                            nc.tensor.matmul(
                                ps[:, 0:TCH],
                                wsb[:, kc, mt * 128:(mt + 1) * 128],
                                xc[:, kc, :],
                                start=(kc == 0),
                                stop=(kc == KC - 1),
                            )
                        nc.vector.tensor_copy(
                            out=dst[tcb][:, mt, :], in_=ps[:, 0:TCH])

                # V chunks: [tokens 128, dims 256]
                for ti in range(TCH // 128):
                    tt = tcb * (TCH // 128) + ti
                    ps = popool.tile([128, 512], F32, name="ps1", tag="po")
                    for kc in range(KC):
                        nc.tensor.matmul(
                            ps[:, 0:DG],
                            xc[:, kc, ti * 128:(ti + 1) * 128],
                            wv_sb[:, kc, :],
                            start=(kc == 0),
                            stop=(kc == KC - 1),
                        )
                    nc.vector.tensor_copy(
                        out=Vs[tcb][:, ti, :, 0:D],
                        in_=ps[:, 0:DG].rearrange("p (h d) -> p h d", h=HG))

            # ---- phase 2: attention (scores transposed, head pairs) ----
            # OT2[p, hm, q]: partition p = 64*j + d for head h = 2*hm + j.
            # This matches wo_sb's row layout so fc_out contracts K=128/pair.
            OT2 = otpool.tile([128, 2, SEQ], MD)
            QC = 1024  # q-chunk: one [128, QC] psum = 2 banks, one exp inst

            for hm in range(2):
                for qc in range(SEQ // QC):
                    qs = slice(qc * QC, (qc + 1) * QC)
                    po = [popool.tile([D + 1, QC], F32, name="po", tag="po")
                          for _ in range(2)]
                    for m in range(SEQ // 128):
                        es = []
                        for j in range(2):  # paired heads -> concurrent MMs
                            ps = pspool.tile([128, QC], F32)
                            for ha in range(QC // 512):
                                nc.tensor.matmul(
                                    ps[:, ha * 512:(ha + 1) * 512],
                                    KTs[m // 4][j * D:(j + 1) * D, hm,
                                                (m % 4) * 128:
                                                (m % 4 + 1) * 128],
                                    QTs[2 * qc + ha][j * D:(j + 1) * D, hm, :],
                                    start=True,
                                    stop=True,
                                )
                            e = epool.tile([128, QC], MD)
                            nc.scalar.activation(
                                out=e, in_=ps,
                                func=mybir.ActivationFunctionType.Exp,
                                scale=1.0 / np.sqrt(D),
                            )
                            es.append(e)
                        for j in range(2):
                            for ha in range(QC // 512):
                                nc.tensor.matmul(
                                    po[j][:, ha * 512:(ha + 1) * 512],
                                    Vs[m // 4][:, m % 4, 2 * hm + j, :],
                                    es[j][:, ha * 512:(ha + 1) * 512],
                                    start=(m == 0),
                                    stop=(m == SEQ // 128 - 1),
                                )
                    for j in range(2):
                        h = 2 * hm + j
                        st = stpool.tile([D + 1, QC], F32)
                        nc.vector.tensor_copy(out=st, in_=po[j])
                        ot_dst = OT2[j * D:(j + 1) * D, hm, qs]
                        if MD == BF16:
                            nc.gpsimd.dma_start(out=ot_dst, in_=st[0:D, :])
                        elif MD == F32:
                            nc.sync.dma_start(out=ot_dst, in_=st[0:D, :])
                        else:
                            nc.sync.dma_start(
                                out=ot_dst, in_=st[0:D, :].bitcast(MD))
                        nc.sync.dma_start(
                            out=den_dram[h:h + 1, qs], in_=st[D:D + 1, :])

            # reciprocal, reshaped to use all 128 partitions (free dim 64)
            rsm = denpool.tile([128, HG * SEQ // 128], F32)
            den_r = den_dram.rearrange("h (a b) -> (h a) b", a=32)
            rden_r = rden_dram.rearrange("h (a b) -> (h a) b", a=32)
            nc.sync.dma_start(out=rsm, in_=den_r)
            nc.vector.reciprocal(out=rsm, in_=rsm)
            nc.sync.dma_start(out=rden_r, in_=rsm)

            # normalize O^T rows by 1/denominator (broadcast across partitions)
            for hm in range(2):
                rb = rbcpool.tile([128, SEQ], F32)
                for j in range(2):
                    nc.sync.dma_start(
                        out=rb[j * D:(j + 1) * D, :],
                        in_=rden_dram[2 * hm + j:2 * hm + j + 1, :]
                        .to_broadcast((D, SEQ)))
                nc.vector.tensor_mul(OT2[:, hm, :], OT2[:, hm, :], rb)

            # ---- phase 3: partial fc_out  y = sum_h O_h @ Wo_h (K=128/pair) ----
            for tt in range(SEQ // 128):
                for nch in range(EMBED // 512):
                    ps = pspool.tile([128, QC], F32)
                    for hm in range(2):
                        nc.tensor.matmul(
                            ps[:, 0:512],
                            OT2[:, hm, tt * 128:(tt + 1) * 128],
                            wo_sb[:, hm, nch * 512:(nch + 1) * 512],
                            start=(hm == 0),
                            stop=(hm == 1),
                        )
                    ys = ypool.tile([128, 512], F32)
                    nc.vector.tensor_copy(out=ys, in_=ps[:, 0:512])
                    nc.sync.dma_start(
                        out=y[tt * 128:(tt + 1) * 128, nch * 512:(nch + 1) * 512],
                        in_=ys)

    nc.compile()
    return nc


def shard_inputs(x, Wv, Wk, Wq, Wo):
    """Build the 8 per-core input maps."""
    in_maps = []
    for c in range(NCORES):
        n, g = divmod(c, GROUPS)
        cols = slice(g * DG, (g + 1) * DG)
        wire = np.float32
        if MM_DTYPE == BF16:
            import ml_dtypes
            wire = ml_dtypes.bfloat16
        in_maps.append({
            "xT": np.ascontiguousarray(np.asarray(x[n], np.float32).T).astype(wire),
            "wq": np.ascontiguousarray(np.asarray(Wq, np.float32)[:, cols]).astype(wire),
            "wk": np.ascontiguousarray(np.asarray(Wk, np.float32)[:, cols]).astype(wire),
            "wv": np.ascontiguousarray(np.asarray(Wv, np.float32)[:, cols]).astype(wire),
            "wo": np.ascontiguousarray(np.asarray(Wo, np.float32)[cols, :]).astype(wire),
        })
    return in_maps


def kernel(x, Wv, Wk, Wq, Wo, bo):
    global LAST_RESULTS
    x = np.asarray(x, np.float32)
    in_maps = shard_inputs(x, Wv, Wk, Wq, Wo)

    if "nc" not in _CACHED_NC:
        _CACHED_NC["nc"] = build_nc()
    nc = _CACHED_NC["nc"]

    trace = os.environ.get("MHA_TRACE", "0") == "1"
    res = bass_utils.run_bass_kernel_spmd(
        nc, in_maps, core_ids=list(range(NCORES)), trace=trace)
    LAST_RESULTS = res

    bo = np.asarray(bo, np.float32)
    out = np.empty((NB, SEQ, EMBED), np.float32)
    for n in range(NB):
        acc = res.results[n * GROUPS]["y"].astype(np.float32).copy()
        for g in range(1, GROUPS):
            acc += res.results[n * GROUPS + g]["y"]
        out[n] = acc + bo[None, :]
    return out



# revision 10
# speedup vs baseline: 1.0143x; 1.0143x over previous
"""Multi-head self-attention Trainium2 kernel (8-core SPMD, full IO).

Problem: x:(2,2048,1024) f32; Wq/Wk/Wv/Wo:(1024,1024); bo:(1024,)
  out = softmax((xWq)(xWk)^T / 8) (xWv) reshaped @ Wo + bo

Sharding: data parallel on batch N=2 x tensor parallel on 16 heads in
4 groups of 4 heads.  Core c handles batch c//4, heads [4*(c%4), 4*(c%4)+4).
Each core computes a partial fc_out product (2048,1024); the host sums the
4 head-group partials per batch and adds the bias.

v2 design (vs the phase-sequential baseline):
  - q-chunk-outer pipeline: per 512-token q-chunk, attention -> denominator
    reciprocal -> normalize -> fc_out -> y DMA, all overlapped with the next
    q-chunk's attention.  No global post-attention stall.
  - projections interleaved INTO the first q-chunk's m-loop so the scalar
    engine (exp) starts ~15us in instead of ~65us, and the PE stays
    continuously busy (p-state ramp: PE reaches 2.4GHz only after ~3us of
    gap-free execution).
  - denominator: ones-column of V makes row 64 of the O^T psum the softmax
    denominator; per q-chunk it is reciprocal'd on DVE (reciprocal_approx_fast)
    and broadcast across partitions with a K=1 ones matmul on the PE --
    no DRAM bounce.
  - scores are computed TRANSPOSED (S^T[k,q]) so exp runs on ACT out of psum
    [128,1024] (one inst per head-pair per k-chunk) and AV contracts k on
    partitions.  No max subtraction (scores ~N(0,1) after /8 scaling).
"""

import os

import numpy as np

import concourse.bass as bass
import concourse.tile as tile
from concourse import bacc, mybir
from concourse import bass_utils

F32 = mybir.dt.float32
F32R = mybir.dt.float32r
BF16 = mybir.dt.bfloat16

EMBED = 1024
SEQ = 2048
NB = 2  # batch
HEADS = 16
D = 64  # head dim
NCORES = 8
GROUPS = 4  # head groups (tensor parallel)
HG = HEADS // GROUPS  # heads per core = 4
DG = HG * D  # dims per core = 256
KC = EMBED // 128  # 8 contraction chunks for projections
TCH = 512  # token chunk (projection granularity == q-chunk granularity)
NT = SEQ // TCH  # 4 chunks
QC = 512  # q tokens per attention block

_MM_DTYPE_NAME = "bfloat16"

# set by run_cores(); test.py reads exec_time_ns from here
LAST_RESULTS = None
_CACHED_NC = {}


def build_nc():
    nc = bacc.Bacc("TRN2", target_bir_lowering=False, debug=False,
                   num_devices=NCORES)

    xT = nc.dram_tensor("xT", (EMBED, SEQ), BF16, kind="ExternalInput").ap()
    wq = nc.dram_tensor("wq", (EMBED, DG), BF16, kind="ExternalInput").ap()
    wk = nc.dram_tensor("wk", (EMBED, DG), BF16, kind="ExternalInput").ap()
    wv = nc.dram_tensor("wv", (EMBED, DG), BF16, kind="ExternalInput").ap()
    wo = nc.dram_tensor("wo", (DG, EMBED), BF16, kind="ExternalInput").ap()
    sel = nc.dram_tensor("sel", (HG, 2 * 128), F32, kind="ExternalInput").ap()
    y = nc.dram_tensor("y", (SEQ, EMBED), F32, kind="ExternalOutput").ap()

    with tile.TileContext(nc) as tc:
        with (
            tc.tile_pool(name="weights", bufs=1) as wpool,
            tc.tile_pool(name="qk", bufs=1) as qkpool,
            tc.tile_pool(name="vpool", bufs=1) as vpool,
            tc.tile_pool(name="otpool", bufs=1) as otpool,
            tc.tile_pool(name="xchunk", bufs=1) as xpool,
            tc.tile_pool(name="epool", bufs=4) as epool,
            tc.tile_pool(name="stage", bufs=2) as stpool,
            tc.tile_pool(name="den", bufs=1) as denpool,
            tc.tile_pool(name="ystage", bufs=3) as ypool,
            tc.tile_pool(name="ps_sc", bufs=2, space="PSUM") as psB,  # scores
            tc.tile_pool(name="ps_po", bufs=1, space="PSUM") as psA,  # O^T acc
            tc.tile_pool(name="ps_mc", bufs=2, space="PSUM") as psC,  # proj/fc/rb
        ):
            # ---- persistent tiles ----
            wq_sb = wpool.tile([128, KC, DG], BF16, name="wq_sb", tag="wq")
            wk_sb = wpool.tile([128, KC, DG], BF16, name="wk_sb", tag="wk")
            wv_sb = wpool.tile([128, KC, DG], BF16, name="wv_sb", tag="wv")
            wo_sb = wpool.tile([128, DG // 128, EMBED], BF16, name="wo_sb",
                               tag="wo")
            QTs = [qkpool.tile([128, 2, TCH], BF16, name=f"qt{t}", tag=f"qt{t}")
                   for t in range(NT)]
            KTs = [qkpool.tile([128, 2, TCH], BF16, name=f"kt{t}", tag=f"kt{t}")
                   for t in range(NT)]
            Vs = [vpool.tile([128, 4, HG, D + 1], BF16, name=f"v{t}",
                             tag=f"v{t}") for t in range(NT)]
            OT2 = otpool.tile([128, 2, SEQ], BF16, name="ot2", tag="ot2")
            xcs = [xpool.tile([128, KC, TCH], BF16, name=f"xc{t}", tag=f"xc{t}")
                   for t in range(NT)]
            den_sb = denpool.tile([HG, SEQ], F32, name="den_sb", tag="den")
            rden_sb = denpool.tile([HG, SEQ], F32, name="rden_sb", tag="rden")
            # one-hot selector: sel[h, hm*128 + p] = 1 iff h == 2*hm + p//64.
            # K=4 matmul sel.T @ rden broadcasts each head's 1/den row onto
            # that head's 64 dim-partitions of rb.
            sel_sb = denpool.tile([HG, 2, 128], F32, name="sel_sb", tag="sel")
            warm_in = denpool.tile([1, 8], F32, name="warm_in", tag="wi")
            warm_out = denpool.tile([1, 8], BF16, name="warm_out", tag="wo2")

            # ---- input DMAs (ordered by first use) ----
            xTr = xT.rearrange("(c p) s -> p c s", p=128)
            nc.sync.dma_start(out=wk_sb,
                              in_=wk.rearrange("(c p) n -> p c n", p=128))
            nc.sync.dma_start(out=xcs[0], in_=xTr[:, :, 0:TCH])
            nc.sync.dma_start(out=wv_sb,
                              in_=wv.rearrange("(c p) n -> p c n", p=128))
            nc.sync.dma_start(out=wq_sb,
                              in_=wq.rearrange("(c p) n -> p c n", p=128))
            for t in range(1, NT):
                nc.sync.dma_start(out=xcs[t],
                                  in_=xTr[:, :, t * TCH:(t + 1) * TCH])
            nc.sync.dma_start(out=wo_sb,
                              in_=wo.rearrange("(c p) n -> p c n", p=128))

            # constants: ones column of V (denominator trick), ones row for
            # the reciprocal broadcast matmul; ACT exp-table warmup
            for t in range(NT):
                nc.vector.memset(Vs[t][:, :, :, D:D + 1], 1.0)
            nc.sync.dma_start(
                out=sel_sb, in_=sel.rearrange("h (c p) -> h c p", c=2))
            nc.vector.memset(warm_in, 0.0)
            nc.scalar.activation(out=warm_out, in_=warm_in,
                                 func=mybir.ActivationFunctionType.Exp,
                                 scale=1.0)

            # ---- projection pieces (emitted interleaved with attention) ----
            def emit_qk(wsb, dst, t, mt):
                ps = psC.tile([128, 512], F32, name="pp", tag="pc")
                for kc in range(KC):
                    nc.tensor.matmul(
                        ps,
                        wsb[:, kc, mt * 128:(mt + 1) * 128],
                        xcs[t][:, kc, :],
                        start=(kc == 0),
                        stop=(kc == KC - 1),
                    )
                nc.vector.tensor_copy(out=dst[t][:, mt, :], in_=ps)

            def emit_v(t, ti):
                ps = psC.tile([128, 512], F32, name="pv", tag="pc")
                for kc in range(KC):
                    nc.tensor.matmul(
                        ps[:, 0:DG],
                        xcs[t][:, kc, ti * 128:(ti + 1) * 128],
                        wv_sb[:, kc, :],
                        start=(kc == 0),
                        stop=(kc == KC - 1),
                    )
                nc.vector.tensor_copy(
                    out=Vs[t][:, ti, :, 0:D],
                    in_=ps[:, 0:DG].rearrange("p (h d) -> p h d", h=HG))

            def emit_piece(p):
                kind, t, idx = p
                if kind == "K":
                    emit_qk(wk_sb, KTs, t, idx)
                elif kind == "Q":
                    emit_qk(wq_sb, QTs, t, idx)
                else:
                    emit_v(t, idx)

            # chunk 0 projections up front (K first: scores need K + Q)
            for p in (("K", 0, 0), ("K", 0, 1), ("Q", 0, 0), ("Q", 0, 1),
                      ("V", 0, 0), ("V", 0, 1), ("V", 0, 2), ("V", 0, 3)):
                emit_piece(p)

            # remaining projections scheduled just-in-time inside (qc0, hm)
            # m-loops: chunk t's K must precede scores m=4t, V_ti before AV
            # m=4t+ti; Q chunks are needed from qc1 on.
            schedule = {
                (0, 0): {
                    1: [("K", 1, 0)], 2: [("K", 1, 1)],
                    3: [("V", 1, 0)], 4: [("V", 1, 1)],
                    5: [("V", 1, 2), ("K", 2, 0)],
                    6: [("V", 1, 3), ("K", 2, 1)],
                    7: [("V", 2, 0)], 8: [("V", 2, 1)],
                    9: [("V", 2, 2), ("K", 3, 0)],
                    10: [("V", 2, 3), ("K", 3, 1)],
                    11: [("V", 3, 0)], 12: [("V", 3, 1)],
                    13: [("V", 3, 2)], 14: [("V", 3, 3)],
                },
                (0, 1): {
                    0: [("Q", 1, 0)], 2: [("Q", 1, 1)],
                },
                (1, 0): {
                    1: [("Q", 2, 0)], 3: [("Q", 2, 1)],
                },
                (1, 1): {
                    1: [("Q", 3, 0)], 3: [("Q", 3, 1)],
                },
            }

            # ---- attention + per-q-chunk epilogue ----
            for qcb in range(SEQ // QC):
                qs = slice(qcb * QC, (qcb + 1) * QC)
                for hm in range(2):
                    po = [psA.tile([D + 1, QC], F32, name=f"po{j}",
                                   tag=f"po{j}") for j in range(2)]
                    for m in range(SEQ // 128):
                        for p in schedule.get((qcb, hm), {}).get(m, []):
                            emit_piece(p)
                        ps = psB.tile([128, 2 * QC], F32, name="sc", tag="sc")
                        for j in range(2):
                            nc.tensor.matmul(
                                ps[:, j * QC:(j + 1) * QC],
                                KTs[m // 4][j * D:(j + 1) * D, hm,
                                            (m % 4) * 128:(m % 4 + 1) * 128],
                                QTs[qcb][j * D:(j + 1) * D, hm, :],
                                start=True,
                                stop=True,
                            )
                        e = epool.tile([128, 2 * QC], BF16, name="e", tag="e")
                        nc.scalar.activation(
                            out=e, in_=ps,
                            func=mybir.ActivationFunctionType.Exp,
                            scale=1.0 / np.sqrt(D),
                        )
                        for j in range(2):
                            nc.tensor.matmul(
                                po[j],
                                Vs[m // 4][:, m % 4, 2 * hm + j, :],
                                e[:, j * QC:(j + 1) * QC],
                                start=(m == 0),
                                stop=(m == SEQ // 128 - 1),
                            )
                    # drain O^T + denominator rows for this (q-chunk, pair)
                    for j in range(2):
                        st = stpool.tile([D + 1, QC], F32, name="st", tag="st")
                        nc.vector.tensor_copy(out=st, in_=po[j])
                        nc.sync.dma_start(
                            out=den_sb[2 * hm + j:2 * hm + j + 1, qs],
                            in_=st[D:D + 1, :])
                        # casting DMA (f32 -> bf16, partition remap) via
                        # software DGE on gpsimd
                        nc.gpsimd.dma_start(
                            out=OT2[j * D:(j + 1) * D, hm, qs],
                            in_=st[0:D, :])

                # 1/denominator (DVE, ~51 ULP is plenty: den ~ 2048*E[exp])
                nc.vector.reciprocal_approx_fast(
                    out=rden_sb[:, qs], in_=den_sb[:, qs])
                # broadcast 1/den across the 64 dim-partitions of each head
                # with a K=1 ones matmul, then normalize O^T in place
                for hm in range(2):
                    rb = psC.tile([128, QC], F32, name="rb", tag="pc")
                    nc.tensor.matmul(
                        rb,
                        sel_sb[:, hm, :],
                        rden_sb[:, qs],
                        start=True,
                        stop=True,
                    )
                    nc.vector.tensor_mul(OT2[:, hm, qs], OT2[:, hm, qs], rb)

                # partial fc_out for this q-chunk: y = sum_hm O^T_hm.T @ Wo_hm
                for tt in range(QC // 128):
                    trow = qcb * QC + tt * 128
                    for nch in range(EMBED // 512):
                        ps = psC.tile([128, 512], F32, name="fo", tag="pc")
                        for hm in range(2):
                            nc.tensor.matmul(
                                ps,
                                OT2[:, hm, trow:trow + 128],
                                wo_sb[:, hm, nch * 512:(nch + 1) * 512],
                                start=(hm == 0),
                                stop=(hm == 1),
                            )
                        ys = ypool.tile([128, 512], F32, name="ys", tag="ys")
                        nc.vector.tensor_copy(out=ys, in_=ps)
                        nc.sync.dma_start(
                            out=y[trow:trow + 128,
                                  nch * 512:(nch + 1) * 512],
                            in_=ys)

    nc.compile()
    return nc


def shard_inputs(x, Wv, Wk, Wq, Wo):
    """Build the 8 per-core input maps."""
    import ml_dtypes
    wire = ml_dtypes.bfloat16
    # one-hot broadcast selector: sel[h, hm*128 + p] = 1 iff h == 2*hm + p//64
    sel_np = np.zeros((HG, 2 * 128), np.float32)
    for hm in range(2):
        for j in range(2):
            sel_np[2 * hm + j, hm * 128 + j * D:hm * 128 + (j + 1) * D] = 1.0
    in_maps = []
    for c in range(NCORES):
        n, g = divmod(c, GROUPS)
        cols = slice(g * DG, (g + 1) * DG)
        in_maps.append({
            "sel": sel_np,
            "xT": np.ascontiguousarray(
                np.asarray(x[n], np.float32).T).astype(wire),
            "wq": np.ascontiguousarray(
                np.asarray(Wq, np.float32)[:, cols]).astype(wire),
            "wk": np.ascontiguousarray(
                np.asarray(Wk, np.float32)[:, cols]).astype(wire),
            "wv": np.ascontiguousarray(
                np.asarray(Wv, np.float32)[:, cols]).astype(wire),
            "wo": np.ascontiguousarray(
                np.asarray(Wo, np.float32)[cols, :]).astype(wire),
        })
    return in_maps


def kernel(x, Wv, Wk, Wq, Wo, bo):
    global LAST_RESULTS
    x = np.asarray(x, np.float32)
    in_maps = shard_inputs(x, Wv, Wk, Wq, Wo)

    if "nc" not in _CACHED_NC:
        _CACHED_NC["nc"] = build_nc()
    nc = _CACHED_NC["nc"]

    trace = os.environ.get("MHA_TRACE", "0") == "1"
    res = bass_utils.run_bass_kernel_spmd(
        nc, in_maps, core_ids=list(range(NCORES)), trace=trace)
    LAST_RESULTS = res

    bo = np.asarray(bo, np.float32)
    out = np.empty((NB, SEQ, EMBED), np.float32)
    for n in range(NB):
        acc = res.results[n * GROUPS]["y"].astype(np.float32).copy()
        for g in range(1, GROUPS):
            acc += res.results[n * GROUPS + g]["y"]
        out[n] = acc + bo[None, :]
    return out


# revision 14
# speedup vs baseline: 1.2189x; 1.2016x over previous
"""Multi-head self-attention Trainium2 kernel (8-core SPMD, full IO).

Problem: x:(2,2048,1024) f32; Wq/Wk/Wv/Wo:(1024,1024); bo:(1024,)
  out = softmax((xWq)(xWk)^T / 8) (xWv) reshaped @ Wo + bo

Sharding: data parallel on batch N=2 x tensor parallel on 16 heads in
4 groups of 4 heads.  Core c handles batch c//4, heads [4*(c%4), 4*(c%4)+4).
Each core computes a partial fc_out product (2048,1024); the host sums the
4 head-group partials per batch and adds the bias.

v2 design (vs the phase-sequential baseline):
  - q-chunk-outer pipeline: per 512-token q-chunk, attention -> denominator
    reciprocal -> normalize -> fc_out -> y DMA, all overlapped with the next
    q-chunk's attention.  No global post-attention stall.
  - projections interleaved INTO the first q-chunk's m-loop so the scalar
    engine (exp) starts ~15us in instead of ~65us, and the PE stays
    continuously busy (p-state ramp: PE reaches 2.4GHz only after ~3us of
    gap-free execution).
  - denominator: ones-column of V makes row 64 of the O^T psum the softmax
    denominator; per q-chunk it is reciprocal'd on DVE (reciprocal_approx_fast)
    and broadcast across partitions with a K=1 ones matmul on the PE --
    no DRAM bounce.
  - scores are computed TRANSPOSED (S^T[k,q]) so exp runs on ACT out of psum
    [128,1024] (one inst per head-pair per k-chunk) and AV contracts k on
    partitions.  No max subtraction (scores ~N(0,1) after /8 scaling).
"""

import os

import numpy as np

import concourse.bass as bass
import concourse.tile as tile
from concourse import bacc, mybir
from concourse import bass_utils

F32 = mybir.dt.float32
F32R = mybir.dt.float32r
BF16 = mybir.dt.bfloat16

EMBED = 1024
SEQ = 2048
NB = 2  # batch
HEADS = 16
D = 64  # head dim
NCORES = 8
GROUPS = 4  # head groups (tensor parallel)
HG = HEADS // GROUPS  # heads per core = 4
DG = HG * D  # dims per core = 256
KC = EMBED // 128  # 8 contraction chunks for projections
TCH = 512  # token chunk (projection granularity == q-chunk granularity)
NT = SEQ // TCH  # 4 chunks
QC = 512  # q tokens per attention block

_MM_DTYPE_NAME = "bfloat16"

# set by run_cores(); test.py reads exec_time_ns from here
LAST_RESULTS = None
_CACHED_NC = {}


def build_nc():
    nc = bacc.Bacc("TRN2", target_bir_lowering=False, debug=False,
                   num_devices=NCORES)

    xT = nc.dram_tensor("xT", (EMBED, SEQ), BF16, kind="ExternalInput").ap()
    wq = nc.dram_tensor("wq", (EMBED, DG), BF16, kind="ExternalInput").ap()
    wk = nc.dram_tensor("wk", (EMBED, DG), BF16, kind="ExternalInput").ap()
    wv = nc.dram_tensor("wv", (EMBED, DG), BF16, kind="ExternalInput").ap()
    wo = nc.dram_tensor("wo", (DG, EMBED), BF16, kind="ExternalInput").ap()
    sel = nc.dram_tensor("sel", (HG, 2 * 128), F32, kind="ExternalInput").ap()
    y = nc.dram_tensor("y", (SEQ, EMBED), F32, kind="ExternalOutput").ap()

    with tile.TileContext(nc) as tc:
        with (
            tc.tile_pool(name="weights", bufs=1) as wpool,
            tc.tile_pool(name="qk", bufs=1) as qkpool,
            tc.tile_pool(name="vpool", bufs=1) as vpool,
            tc.tile_pool(name="otpool", bufs=1) as otpool,
            tc.tile_pool(name="xchunk", bufs=1) as xpool,
            tc.tile_pool(name="epool", bufs=4) as epool,
            tc.tile_pool(name="stage", bufs=2) as stpool,
            tc.tile_pool(name="den", bufs=1) as denpool,
            tc.tile_pool(name="ystage", bufs=3) as ypool,
            tc.tile_pool(name="ps_sc", bufs=2, space="PSUM") as psB,  # scores
            tc.tile_pool(name="ps_po", bufs=1, space="PSUM") as psA,  # O^T acc
            tc.tile_pool(name="ps_mc", bufs=2, space="PSUM") as psC,  # proj/fc/rb
        ):
            # ---- persistent tiles ----
            wq_sb = wpool.tile([128, KC, DG], BF16, name="wq_sb", tag="wq")
            wk_sb = wpool.tile([128, KC, DG], BF16, name="wk_sb", tag="wk")
            wv_sb = wpool.tile([128, KC, DG], BF16, name="wv_sb", tag="wv")
            wo_sb = wpool.tile([128, DG // 128, EMBED], BF16, name="wo_sb",
                               tag="wo")
            QTs = [qkpool.tile([128, 2, TCH], BF16, name=f"qt{t}", tag=f"qt{t}")
                   for t in range(NT)]
            KTs = [qkpool.tile([128, 2, TCH], BF16, name=f"kt{t}", tag=f"kt{t}")
                   for t in range(NT)]
            Vs = [vpool.tile([128, 4, HG, D + 1], BF16, name=f"v{t}",
                             tag=f"v{t}") for t in range(NT)]
            OT2 = otpool.tile([128, 2, SEQ], BF16, name="ot2", tag="ot2")
            xcs = [xpool.tile([128, KC, TCH], BF16, name=f"xc{t}", tag=f"xc{t}")
                   for t in range(NT)]
            den_sb = denpool.tile([HG, SEQ], F32, name="den_sb", tag="den")
            rden_sb = denpool.tile([HG, SEQ], F32, name="rden_sb", tag="rden")
            # one-hot selector: sel[h, hm*128 + p] = 1 iff h == 2*hm + p//64.
            # K=4 matmul sel.T @ rden broadcasts each head's 1/den row onto
            # that head's 64 dim-partitions of rb.
            sel_sb = denpool.tile([HG, 2, 128], F32, name="sel_sb", tag="sel")
            warm_in = denpool.tile([1, 8], F32, name="warm_in", tag="wi")
            warm_out = denpool.tile([1, 8], BF16, name="warm_out", tag="wo2")

            dm_l = denpool.tile([128, 128], BF16, name="dm_l", tag="dml")
            dm_r = denpool.tile([128, 512], BF16, name="dm_r", tag="dmr")

            # ---- input DMAs, split across the two HWDGE queues ----
            # sync queue: the K0/Q0 critical path (x0 in kc-pieces so the K0
            # projection chain starts mid-transfer).  scalar queue: the rest.
            xTr = xT.rearrange("(c p) s -> p c s", p=128)
            nc.sync.dma_start(out=wk_sb,
                              in_=wk.rearrange("(c p) n -> p c n", p=128))
            for kc in range(KC):
                nc.sync.dma_start(out=xcs[0][:, kc:kc + 1, :],
                                  in_=xTr[:, kc:kc + 1, 0:TCH])
            nc.sync.dma_start(out=wq_sb,
                              in_=wq.rearrange("(c p) n -> p c n", p=128))
            nc.scalar.dma_start(
                out=sel_sb, in_=sel.rearrange("h (c p) -> h c p", c=2))
            nc.scalar.dma_start(out=wv_sb,
                                in_=wv.rearrange("(c p) n -> p c n", p=128))
            for t in range(1, NT):
                nc.scalar.dma_start(out=xcs[t],
                                    in_=xTr[:, :, t * TCH:(t + 1) * TCH])
            nc.scalar.dma_start(out=wo_sb,
                                in_=wo.rearrange("(c p) n -> p c n", p=128))

            # constants: ones column of V (denominator trick); ACT exp-table
            # warmup; zero tiles for the PE p-state warmup matmuls
            nc.vector.memset(dm_l, 0.0)
            nc.vector.memset(dm_r, 0.0)
            for t in range(NT):
                nc.vector.memset(Vs[t][:, :, :, D:D + 1], 1.0)
            nc.vector.memset(warm_in, 0.0)
            nc.scalar.activation(out=warm_out, in_=warm_in,
                                 func=mybir.ActivationFunctionType.Exp,
                                 scale=1.0)

            # PE p-state warmup: the tensor engine ramps 0.65 -> 1.2 -> 2.4GHz
            # only over ~3us of gap-free execution.  Burn the DMA-prologue
            # time ramping on throwaway matmuls so projections run at speed.
            for _ in range(20):
                psd = psC.tile([128, 512], F32, name="pd", tag="pc")
                nc.tensor.matmul(psd, dm_l, dm_r, start=True, stop=True)

            # ---- projection pieces (emitted interleaved with attention) ----
            def emit_qk(wsb, dst, t, mt):
                ps = psC.tile([128, 512], F32, name="pp", tag="pc")
                for kc in range(KC):
                    nc.tensor.matmul(
                        ps,
                        wsb[:, kc, mt * 128:(mt + 1) * 128],
                        xcs[t][:, kc, :],
                        start=(kc == 0),
                        stop=(kc == KC - 1),
                    )
                nc.vector.tensor_copy(out=dst[t][:, mt, :], in_=ps)

            def emit_v(t, ti):
                ps = psC.tile([128, 512], F32, name="pv", tag="pc")
                for kc in range(KC):
                    nc.tensor.matmul(
                        ps[:, 0:DG],
                        xcs[t][:, kc, ti * 128:(ti + 1) * 128],
                        wv_sb[:, kc, :],
                        start=(kc == 0),
                        stop=(kc == KC - 1),
                    )
                nc.vector.tensor_copy(
                    out=Vs[t][:, ti, :, 0:D],
                    in_=ps[:, 0:DG].rearrange("p (h d) -> p h d", h=HG))

            def emit_recip(qcb):
                qs = slice(qcb * QC, (qcb + 1) * QC)
                nc.vector.reciprocal_approx_fast(
                    out=rden_sb[:, qs], in_=den_sb[:, qs])

            def emit_rb_norm(qcb, hm):
                # broadcast 1/den onto each head's 64 dim-partitions (K=4
                # one-hot matmul), then normalize O^T in place
                qs = slice(qcb * QC, (qcb + 1) * QC)
                rb = psC.tile([128, QC], F32, name="rb", tag="pc")
                nc.tensor.matmul(rb, sel_sb[:, hm, :], rden_sb[:, qs],
                                 start=True, stop=True)
                nc.vector.tensor_mul(OT2[:, hm, qs], OT2[:, hm, qs], rb)

            def emit_fc(qcb, k):
                # one (token-block, out-column-block) piece of the partial
                # fc_out for q-chunk qcb
                tt, nch = divmod(k, EMBED // 512)
                trow = qcb * QC + tt * 128
                ps = psC.tile([128, 512], F32, name="fo", tag="pc")
                for hm in range(2):
                    nc.tensor.matmul(
                        ps,
                        OT2[:, hm, trow:trow + 128],
                        wo_sb[:, hm, nch * 512:(nch + 1) * 512],
                        start=(hm == 0),
                        stop=(hm == 1),
                    )
                ys = ypool.tile([128, 512], F32, name="ys", tag="ys")
                nc.vector.tensor_copy(out=ys, in_=ps)
                nc.sync.dma_start(
                    out=y[trow:trow + 128, nch * 512:(nch + 1) * 512],
                    in_=ys)

            def emit_piece(p):
                kind = p[0]
                if kind == "K":
                    emit_qk(wk_sb, KTs, p[1], p[2])
                elif kind == "Q":
                    emit_qk(wq_sb, QTs, p[1], p[2])
                elif kind == "V":
                    emit_v(p[1], p[2])
                elif kind == "EPI":
                    emit_recip(p[1])
                elif kind == "RB":
                    emit_rb_norm(p[1], p[2])
                elif kind == "FC":
                    emit_fc(p[1], p[2])

            # chunk 0 projections up front (K first: scores need K + Q)
            for p in (("K", 0, 0), ("K", 0, 1), ("Q", 0, 0), ("Q", 0, 1),
                      ("V", 0, 0), ("V", 0, 1), ("V", 0, 2), ("V", 0, 3)):
                emit_piece(p)

            # remaining projections scheduled just-in-time inside (qc0, hm)
            # m-loops: chunk t's K must precede scores m=4t, V_ti before AV
            # m=4t+ti; Q chunks are needed from qc1 on.
            schedule = {
                (0, 0): {
                    1: [("K", 1, 0)], 2: [("K", 1, 1)],
                    3: [("V", 1, 0)], 4: [("V", 1, 1)],
                    5: [("V", 1, 2), ("K", 2, 0)],
                    6: [("V", 1, 3), ("K", 2, 1)],
                    7: [("V", 2, 0)], 8: [("V", 2, 1)],
                    9: [("V", 2, 2), ("K", 3, 0)],
                    10: [("V", 2, 3), ("K", 3, 1)],
                    11: [("V", 3, 0)], 12: [("V", 3, 1)],
                    13: [("V", 3, 2)], 14: [("V", 3, 3)],
                },
                (0, 1): {
                    0: [("Q", 1, 0)], 2: [("Q", 1, 1)],
                },
                # q-chunk qcb's epilogue (reciprocal, normalize, fc_out) is
                # deferred into (qcb+1, hm0)'s m-loop so the PE queue never
                # head-of-line blocks on the DVE/DMA epilogue chain.
                (1, 0): {
                    1: [("EPI", 0), ("Q", 2, 0)],
                    2: [("RB", 0, 0)], 3: [("RB", 0, 1), ("Q", 2, 1)],
                    **{4 + k: [("FC", 0, k)] for k in range(8)},
                },
                (1, 1): {
                    1: [("Q", 3, 0)], 3: [("Q", 3, 1)],
                },
                (2, 0): {
                    1: [("EPI", 1)],
                    2: [("RB", 1, 0)], 3: [("RB", 1, 1)],
                    **{4 + k: [("FC", 1, k)] for k in range(8)},
                },
                (3, 0): {
                    1: [("EPI", 2)],
                    2: [("RB", 2, 0)], 3: [("RB", 2, 1)],
                    **{4 + k: [("FC", 2, k)] for k in range(8)},
                },
            }

            # ---- attention + per-q-chunk epilogue ----
            for qcb in range(SEQ // QC):
                qs = slice(qcb * QC, (qcb + 1) * QC)
                for hm in range(2):
                    po = [psA.tile([D + 1, QC], F32, name=f"po{j}",
                                   tag=f"po{j}") for j in range(2)]
                    for m in range(SEQ // 128):
                        for p in schedule.get((qcb, hm), {}).get(m, []):
                            emit_piece(p)
                        ps = psB.tile([128, 2 * QC], F32, name="sc", tag="sc")
                        for j in range(2):
                            nc.tensor.matmul(
                                ps[:, j * QC:(j + 1) * QC],
                                KTs[m // 4][j * D:(j + 1) * D, hm,
                                            (m % 4) * 128:(m % 4 + 1) * 128],
                                QTs[qcb][j * D:(j + 1) * D, hm, :],
                                start=True,
                                stop=True,
                            )
                        e = epool.tile([128, 2 * QC], BF16, name="e", tag="e")
                        nc.scalar.activation(
                            out=e, in_=ps,
                            func=mybir.ActivationFunctionType.Exp,
                            scale=1.0 / np.sqrt(D),
                        )
                        for j in range(2):
                            nc.tensor.matmul(
                                po[j],
                                Vs[m // 4][:, m % 4, 2 * hm + j, :],
                                e[:, j * QC:(j + 1) * QC],
                                start=(m == 0),
                                stop=(m == SEQ // 128 - 1),
                            )
                    # drain O^T + denominator rows for this (q-chunk, pair);
                    # this frees the po psum banks for the next pair
                    for j in range(2):
                        st = stpool.tile([D + 1, QC], F32, name="st", tag="st")
                        nc.vector.tensor_copy(out=st, in_=po[j])
                        nc.sync.dma_start(
                            out=den_sb[2 * hm + j:2 * hm + j + 1, qs],
                            in_=st[D:D + 1, :])
                        # casting DMA (f32 -> bf16, partition remap) via
                        # software DGE on gpsimd
                        nc.gpsimd.dma_start(
                            out=OT2[j * D:(j + 1) * D, hm, qs],
                            in_=st[0:D, :])

            # tail: last q-chunk's epilogue
            emit_recip(NT - 1)
            for hm in range(2):
                emit_rb_norm(NT - 1, hm)
            for k in range(8):
                emit_fc(NT - 1, k)

    nc.compile()
    return nc


def shard_inputs(x, Wv, Wk, Wq, Wo):
    """Build the 8 per-core input maps."""
    import ml_dtypes
    wire = ml_dtypes.bfloat16
    # one-hot broadcast selector: sel[h, hm*128 + p] = 1 iff h == 2*hm + p//64
    sel_np = np.zeros((HG, 2 * 128), np.float32)
    for hm in range(2):
        for j in range(2):
            sel_np[2 * hm + j, hm * 128 + j * D:hm * 128 + (j + 1) * D] = 1.0
    in_maps = []
    for c in range(NCORES):
        n, g = divmod(c, GROUPS)
        cols = slice(g * DG, (g + 1) * DG)
        in_maps.append({
            "sel": sel_np,
            "xT": np.ascontiguousarray(
                np.asarray(x[n], np.float32).T).astype(wire),
            "wq": np.ascontiguousarray(
                np.asarray(Wq, np.float32)[:, cols]).astype(wire),
            "wk": np.ascontiguousarray(
                np.asarray(Wk, np.float32)[:, cols]).astype(wire),
            "wv": np.ascontiguousarray(
                np.asarray(Wv, np.float32)[:, cols]).astype(wire),
            "wo": np.ascontiguousarray(
                np.asarray(Wo, np.float32)[cols, :]).astype(wire),
        })
    return in_maps


def kernel(x, Wv, Wk, Wq, Wo, bo):
    global LAST_RESULTS
    x = np.asarray(x, np.float32)
    in_maps = shard_inputs(x, Wv, Wk, Wq, Wo)

    if "nc" not in _CACHED_NC:
        _CACHED_NC["nc"] = build_nc()
    nc = _CACHED_NC["nc"]

    trace = os.environ.get("MHA_TRACE", "0") == "1"
    res = bass_utils.run_bass_kernel_spmd(
        nc, in_maps, core_ids=list(range(NCORES)), trace=trace)
    LAST_RESULTS = res

    bo = np.asarray(bo, np.float32)
    out = np.empty((NB, SEQ, EMBED), np.float32)
    for n in range(NB):
        acc = res.results[n * GROUPS]["y"].astype(np.float32).copy()
        for g in range(1, GROUPS):
            acc += res.results[n * GROUPS + g]["y"]
        out[n] = acc + bo[None, :]
    return out


# revision 24
# speedup vs baseline: 1.2657x; 1.0385x over previous
"""Multi-head self-attention Trainium2 kernel (8-core SPMD, full IO).

Problem: x:(2,2048,1024) f32; Wq/Wk/Wv/Wo:(1024,1024); bo:(1024,)
  out = softmax((xWq)(xWk)^T / 8) (xWv) reshaped @ Wo + bo

Sharding: data parallel on batch N=2 x tensor parallel on 16 heads in
4 groups of 4 heads.  Core c handles batch c//4, heads [4*(c%4), 4*(c%4)+4).
Each core computes a partial fc_out product (2048,1024); the host sums the
4 head-group partials per batch and adds the bias.

v2 design (vs the phase-sequential baseline):
  - q-chunk-outer pipeline: per 512-token q-chunk, attention -> denominator
    reciprocal -> normalize -> fc_out -> y DMA, all overlapped with the next
    q-chunk's attention.  No global post-attention stall.
  - projections interleaved INTO the first q-chunk's m-loop so the scalar
    engine (exp) starts ~15us in instead of ~65us, and the PE stays
    continuously busy (p-state ramp: PE reaches 2.4GHz only after ~3us of
    gap-free execution).
  - denominator: ones-column of V makes row 64 of the O^T psum the softmax
    denominator; per q-chunk it is reciprocal'd on DVE (reciprocal_approx_fast)
    and broadcast across partitions with a K=1 ones matmul on the PE --
    no DRAM bounce.
  - scores are computed TRANSPOSED (S^T[k,q]) so exp runs on ACT out of psum
    [128,1024] (one inst per head-pair per k-chunk) and AV contracts k on
    partitions.  No max subtraction (scores ~N(0,1) after /8 scaling).
"""

import os

import numpy as np

import concourse.bass as bass
import concourse.tile as tile
from concourse import bacc, mybir
from concourse import bass_utils

F32 = mybir.dt.float32
F32R = mybir.dt.float32r
BF16 = mybir.dt.bfloat16

EMBED = 1024
SEQ = 2048
NB = 2  # batch
HEADS = 16
D = 64  # head dim
NCORES = 8
GROUPS = 4  # head groups (tensor parallel)
HG = HEADS // GROUPS  # heads per core = 4
DG = HG * D  # dims per core = 256
KC = EMBED // 128  # 8 contraction chunks for projections
TCH = 512  # token chunk (projection granularity == q-chunk granularity)
NT = SEQ // TCH  # 4 chunks
QC = 512  # q tokens per attention block

_MM_DTYPE_NAME = "bfloat16"

# set by run_cores(); test.py reads exec_time_ns from here
LAST_RESULTS = None
_CACHED_NC = {}


def build_nc():
    nc = bacc.Bacc("TRN2", target_bir_lowering=False, debug=False,
                   num_devices=NCORES)

    xT = nc.dram_tensor("xT", (EMBED, SEQ), BF16, kind="ExternalInput").ap()
    wq = nc.dram_tensor("wq", (EMBED, DG), BF16, kind="ExternalInput").ap()
    wk = nc.dram_tensor("wk", (EMBED, DG), BF16, kind="ExternalInput").ap()
    wv = nc.dram_tensor("wv", (EMBED, DG), BF16, kind="ExternalInput").ap()
    wo = nc.dram_tensor("wo", (DG, EMBED), BF16, kind="ExternalInput").ap()
    y = nc.dram_tensor("y", (SEQ, EMBED), F32, kind="ExternalOutput").ap()

    with tile.TileContext(nc) as tc:
        with (
            tc.tile_pool(name="weights", bufs=1) as wpool,
            tc.tile_pool(name="qk", bufs=1) as qkpool,
            tc.tile_pool(name="vpool", bufs=1) as vpool,
            tc.tile_pool(name="otpool", bufs=1) as otpool,
            tc.tile_pool(name="xchunk", bufs=1) as xpool,
            tc.tile_pool(name="epool", bufs=4) as epool,
            tc.tile_pool(name="stage", bufs=2) as stpool,
            tc.tile_pool(name="rbp", bufs=4) as rbpool,
            tc.tile_pool(name="den", bufs=1) as denpool,
            tc.tile_pool(name="ystage", bufs=3) as ypool,
            tc.tile_pool(name="ps_sc", bufs=2, space="PSUM") as psB,  # scores
            tc.tile_pool(name="ps_po", bufs=1, space="PSUM") as psA,  # O^T acc
            tc.tile_pool(name="ps_mc", bufs=2, space="PSUM") as psC,  # proj/fc/rb
        ):
            # ---- persistent tiles ----
            wq_sb = wpool.tile([128, KC, DG], BF16, name="wq_sb", tag="wq")
            wk_sb = wpool.tile([128, KC, DG], BF16, name="wk_sb", tag="wk")
            wv_sb = wpool.tile([128, KC, DG], BF16, name="wv_sb", tag="wv")
            wo_sb = wpool.tile([128, DG // 128, EMBED], BF16, name="wo_sb",
                               tag="wo")
            QTs = [qkpool.tile([128, 2, TCH], BF16, name=f"qt{t}", tag=f"qt{t}")
                   for t in range(NT)]
            KTs = [qkpool.tile([128, 2, TCH], BF16, name=f"kt{t}", tag=f"kt{t}")
                   for t in range(NT)]
            Vs = [vpool.tile([128, 4, HG, D + 1], BF16, name=f"v{t}",
                             tag=f"v{t}") for t in range(NT)]
            OT2 = otpool.tile([128, 2, SEQ], BF16, name="ot2", tag="ot2")
            xcs = [xpool.tile([128, KC, TCH], BF16, name=f"xc{t}", tag=f"xc{t}")
                   for t in range(NT)]
            # ones row for the K=1 reciprocal-broadcast matmul
            ones_b = denpool.tile([1, D], BF16, name="ones_b", tag="ones")
            warm_in = denpool.tile([1, 8], F32, name="warm_in", tag="wi")
            warm_out = denpool.tile([1, 8], BF16, name="warm_out", tag="wo2")

            dm_l = denpool.tile([128, 128], BF16, name="dm_l", tag="dml")
            dm_r = denpool.tile([128, 512], BF16, name="dm_r", tag="dmr")

            # ---- input DMAs, split across the two HWDGE queues ----
            # x0 kc-pieces alternate between queues so the K0 projection
            # chain starts mid-transfer; weights interleave by first use.
            xTr = xT.rearrange("(c p) s -> p c s", p=128)
            nc.sync.dma_start(out=wk_sb,
                              in_=wk.rearrange("(c p) n -> p c n", p=128))
            nc.scalar.dma_start(out=wq_sb,
                                in_=wq.rearrange("(c p) n -> p c n", p=128))
            for kc in range(KC):
                q = nc.sync if kc % 2 == 0 else nc.scalar
                q.dma_start(out=xcs[0][:, kc:kc + 1, :],
                            in_=xTr[:, kc:kc + 1, 0:TCH])
            nc.scalar.dma_start(out=wv_sb,
                                in_=wv.rearrange("(c p) n -> p c n", p=128))
            for t in range(1, NT):
                q = nc.sync if t % 2 == 1 else nc.scalar
                q.dma_start(out=xcs[t],
                            in_=xTr[:, :, t * TCH:(t + 1) * TCH])
            nc.scalar.dma_start(out=wo_sb,
                                in_=wo.rearrange("(c p) n -> p c n", p=128))

            # constants: ones column 0 of V (denominator lands in po row 0,
            # base partition 0, so the reciprocal runs in place); ACT
            # exp-table warmup; zero tiles for the PE p-state warmup matmuls
            nc.vector.memset(dm_l, 0.0)
            nc.vector.memset(dm_r, 0.0)
            for t in range(NT):
                nc.vector.memset(Vs[t][:, :, :, 0:1], 1.0)
            nc.vector.memset(ones_b, 1.0)
            nc.vector.memset(warm_in, 0.0)
            nc.scalar.activation(out=warm_out, in_=warm_in,
                                 func=mybir.ActivationFunctionType.Exp,
                                 scale=1.0)

            # PE p-state warmup: the tensor engine ramps 0.65 -> 1.2 -> 2.4GHz
            # only over ~3us of gap-free execution.  Burn the DMA-prologue
            # time ramping on throwaway matmuls so projections run at speed.
            for _ in range(20):
                psd = psC.tile([128, 512], F32, name="pd", tag="pc")
                nc.tensor.matmul(psd, dm_l, dm_r, start=True, stop=True)

            # ---- projection pieces (emitted interleaved with attention) ----
            def emit_qk(wsb, dst, t, mt):
                ps = psC.tile([128, 512], F32, name="pp", tag="pc")
                for kc in range(KC):
                    nc.tensor.matmul(
                        ps,
                        wsb[:, kc, mt * 128:(mt + 1) * 128],
                        xcs[t][:, kc, :],
                        start=(kc == 0),
                        stop=(kc == KC - 1),
                    )
                nc.vector.tensor_copy(out=dst[t][:, mt, :], in_=ps)

            def emit_v(t, ti):
                ps = psC.tile([128, 512], F32, name="pv", tag="pc")
                for kc in range(KC):
                    nc.tensor.matmul(
                        ps[:, 0:DG],
                        xcs[t][:, kc, ti * 128:(ti + 1) * 128],
                        wv_sb[:, kc, :],
                        start=(kc == 0),
                        stop=(kc == KC - 1),
                    )
                nc.vector.tensor_copy(
                    out=Vs[t][:, ti, :, 1:D + 1],
                    in_=ps[:, 0:DG].rearrange("p (h d) -> p h d", h=HG))

            rdens = {}  # (qcb, hm) -> [rden_b tiles j=0,1], set at drain time

            def emit_rb_norm(qcb, hm):
                # broadcast each head's 1/den row onto its 64 dim-partitions
                # (two K=1 ones-matmuls), then normalize O^T in place
                qs = slice(qcb * QC, (qcb + 1) * QC)
                rb = psC.tile([128, QC], F32, name="rb", tag="pc")
                for j in range(2):
                    nc.tensor.matmul(rb[j * D:(j + 1) * D, :], ones_b,
                                     rdens[(qcb, hm)][j],
                                     start=True, stop=True)
                nc.vector.tensor_mul(OT2[:, hm, qs], OT2[:, hm, qs], rb)

            def emit_fc(qcb, k):
                # one (token-block, out-column-block) piece of the partial
                # fc_out for q-chunk qcb
                tt, nch = divmod(k, EMBED // 512)
                trow = qcb * QC + tt * 128
                ps = psC.tile([128, 512], F32, name="fo", tag="pc")
                for hm in range(2):
                    nc.tensor.matmul(
                        ps,
                        OT2[:, hm, trow:trow + 128],
                        wo_sb[:, hm, nch * 512:(nch + 1) * 512],
                        start=(hm == 0),
                        stop=(hm == 1),
                    )
                ys = ypool.tile([128, 512], F32, name="ys", tag="ys")
                nc.vector.tensor_copy(out=ys, in_=ps)
                nc.sync.dma_start(
                    out=y[trow:trow + 128, nch * 512:(nch + 1) * 512],
                    in_=ys)

            def emit_piece(p):
                kind = p[0]
                if kind == "K":
                    emit_qk(wk_sb, KTs, p[1], p[2])
                elif kind == "Q":
                    emit_qk(wq_sb, QTs, p[1], p[2])
                elif kind == "V":
                    emit_v(p[1], p[2])
                elif kind == "RB":
                    emit_rb_norm(p[1], p[2])
                elif kind == "FC":
                    emit_fc(p[1], p[2])

            # minimal prologue: just enough for (qc0, hm0) scores + first AV
            for p in (("K", 0, 0), ("K", 0, 1), ("Q", 0, 0), ("V", 0, 0)):
                emit_piece(p)

            # remaining projections just-in-time inside (qc0, hm) m-loops
            # (chunk t's K before scores m=4t, V_ti before AV m=4t+ti);
            # q-chunk qcb's epilogue (normalize-broadcast RB, fc_out FC) is
            # deferred into qcb+1's m-loops so the PE queue never head-of-line
            # blocks on the DVE/DMA epilogue chain.
            schedule = {
                (0, 0): {
                    0: [("V", 0, 1)],
                    1: [("V", 0, 2), ("K", 1, 0)],
                    2: [("V", 0, 3), ("K", 1, 1)],
                    3: [("V", 1, 0)],
                    4: [("V", 1, 1), ("K", 2, 0)],
                    5: [("V", 1, 2), ("K", 2, 1)],
                    6: [("V", 1, 3)],
                    7: [("V", 2, 0), ("K", 3, 0)],
                    8: [("V", 2, 1), ("K", 3, 1)],
                    9: [("V", 2, 2)], 10: [("V", 2, 3)],
                    11: [("V", 3, 0)], 12: [("V", 3, 1)],
                    13: [("V", 3, 2)], 14: [("V", 3, 3)],
                    15: [("Q", 0, 1)],
                },
                (0, 1): {
                    1: [("Q", 1, 0)], 3: [("Q", 1, 1)],
                },
                (1, 0): {
                    1: [("RB", 0, 0)],
                    3: [("RB", 0, 1), ("Q", 2, 0)],
                    4: [("FC", 0, 0)], 5: [("FC", 0, 1)],
                    6: [("Q", 2, 1)],
                    7: [("FC", 0, 2)], 8: [("FC", 0, 3)],
                },
                (1, 1): {
                    1: [("Q", 3, 0)], 2: [("FC", 0, 4)],
                    3: [("Q", 3, 1)], 4: [("FC", 0, 5)],
                    5: [("FC", 0, 6)], 6: [("FC", 0, 7)],
                },
                (2, 0): {
                    1: [("RB", 1, 0)], 3: [("RB", 1, 1)],
                    4: [("FC", 1, 0)], 5: [("FC", 1, 1)],
                    7: [("FC", 1, 2)], 8: [("FC", 1, 3)],
                },
                (2, 1): {
                    2: [("FC", 1, 4)], 3: [("FC", 1, 5)],
                    5: [("FC", 1, 6)], 6: [("FC", 1, 7)],
                },
                (3, 0): {
                    1: [("RB", 2, 0)], 3: [("RB", 2, 1)],
                    4: [("FC", 2, 0)], 5: [("FC", 2, 1)],
                    7: [("FC", 2, 2)], 8: [("FC", 2, 3)],
                },
                (3, 1): {
                    2: [("FC", 2, 4)], 3: [("FC", 2, 5)],
                    5: [("FC", 2, 6)], 6: [("FC", 2, 7)],
                },
            }

            # ---- attention + per-q-chunk epilogue ----
            for qcb in range(SEQ // QC):
                qs = slice(qcb * QC, (qcb + 1) * QC)
                for hm in range(2):
                    po = [psA.tile([D + 1, QC], F32, name=f"po{j}",
                                   tag=f"po{j}") for j in range(2)]
                    for m in range(SEQ // 128):
                        for p in schedule.get((qcb, hm), {}).get(m, []):
                            emit_piece(p)
                        ps = psB.tile([128, 2 * QC], F32, name="sc", tag="sc")
                        for j in range(2):
                            nc.tensor.matmul(
                                ps[:, j * QC:(j + 1) * QC],
                                KTs[m // 4][j * D:(j + 1) * D, hm,
                                            (m % 4) * 128:(m % 4 + 1) * 128],
                                QTs[qcb][j * D:(j + 1) * D, hm, :],
                                start=True,
                                stop=True,
                            )
                        e = epool.tile([128, 2 * QC], BF16, name="e", tag="e")
                        nc.scalar.activation(
                            out=e, in_=ps,
                            func=mybir.ActivationFunctionType.Exp,
                            scale=1.0 / np.sqrt(D),
                        )
                        for j in range(2):
                            nc.tensor.matmul(
                                po[j],
                                Vs[m // 4][:, m % 4, 2 * hm + j, :],
                                e[:, j * QC:(j + 1) * QC],
                                start=(m == 0),
                                stop=(m == SEQ // 128 - 1),
                            )
                    # drain O^T (row 0 = denominator) for this (q-chunk,
                    # pair); the st copies free the po psum banks, then the
                    # denominator is reciprocal'd in place and staged to bf16
                    # for the RB broadcast matmul
                    sts = []
                    for j in range(2):
                        st = stpool.tile([D + 1, QC], F32, name="st", tag="st")
                        nc.vector.tensor_copy(out=st, in_=po[j])
                        sts.append(st)
                    rpair = []
                    for j, st in enumerate(sts):
                        # casting DMA (f32 -> bf16, partition remap) via
                        # software DGE on gpsimd
                        nc.gpsimd.dma_start(
                            out=OT2[j * D:(j + 1) * D, hm, qs],
                            in_=st[1:D + 1, :])
                        nc.vector.reciprocal_approx_fast(
                            out=st[0:1, :], in_=st[0:1, :])
                        rden_b = rbpool.tile([1, QC], BF16, name="rden_b",
                                             tag="rden")
                        nc.vector.tensor_copy(out=rden_b, in_=st[0:1, :])
                        rpair.append(rden_b)
                    rdens[(qcb, hm)] = rpair

            # tail: last q-chunk's epilogue
            for hm in range(2):
                emit_rb_norm(NT - 1, hm)
            for k in range(8):
                emit_fc(NT - 1, k)

    nc.compile()
    return nc


def shard_inputs(x, Wv, Wk, Wq, Wo):
    """Build the 8 per-core input maps."""
    import ml_dtypes
    wire = ml_dtypes.bfloat16
    in_maps = []
    for c in range(NCORES):
        n, g = divmod(c, GROUPS)
        cols = slice(g * DG, (g + 1) * DG)
        in_maps.append({
            "xT": np.ascontiguousarray(
                np.asarray(x[n], np.float32).T).astype(wire),
            "wq": np.ascontiguousarray(
                np.asarray(Wq, np.float32)[:, cols]).astype(wire),
            "wk": np.ascontiguousarray(
                np.asarray(Wk, np.float32)[:, cols]).astype(wire),
            "wv": np.ascontiguousarray(
                np.asarray(Wv, np.float32)[:, cols]).astype(wire),
            "wo": np.ascontiguousarray(
                np.asarray(Wo, np.float32)[cols, :]).astype(wire),
        })
    return in_maps


def kernel(x, Wv, Wk, Wq, Wo, bo):
    global LAST_RESULTS
    x = np.asarray(x, np.float32)
    in_maps = shard_inputs(x, Wv, Wk, Wq, Wo)

    if "nc" not in _CACHED_NC:
        _CACHED_NC["nc"] = build_nc()
    nc = _CACHED_NC["nc"]

    trace = os.environ.get("MHA_TRACE", "0") == "1"
    res = bass_utils.run_bass_kernel_spmd(
        nc, in_maps, core_ids=list(range(NCORES)), trace=trace)
    LAST_RESULTS = res

    bo = np.asarray(bo, np.float32)
    out = np.empty((NB, SEQ, EMBED), np.float32)
    for n in range(NB):
        acc = res.results[n * GROUPS]["y"].astype(np.float32).copy()
        for g in range(1, GROUPS):
            acc += res.results[n * GROUPS + g]["y"]
        out[n] = acc + bo[None, :]
    return out


# revision 27
# speedup vs baseline: 1.2770x; 1.0089x over previous
"""Multi-head self-attention Trainium2 kernel (8-core SPMD, full IO).

Problem: x:(2,2048,1024) f32; Wq/Wk/Wv/Wo:(1024,1024); bo:(1024,)
  out = softmax((xWq)(xWk)^T / 8) (xWv) reshaped @ Wo + bo

Sharding: data parallel on batch N=2 x tensor parallel on 16 heads in
4 groups of 4 heads.  Core c handles batch c//4, heads [4*(c%4), 4*(c%4)+4).
Each core computes a partial fc_out product (2048,1024); the host sums the
4 head-group partials per batch and adds the bias.

v2 design (vs the phase-sequential baseline):
  - q-chunk-outer pipeline: per 512-token q-chunk, attention -> denominator
    reciprocal -> normalize -> fc_out -> y DMA, all overlapped with the next
    q-chunk's attention.  No global post-attention stall.
  - projections interleaved INTO the first q-chunk's m-loop so the scalar
    engine (exp) starts ~15us in instead of ~65us, and the PE stays
    continuously busy (p-state ramp: PE reaches 2.4GHz only after ~3us of
    gap-free execution).
  - denominator: ones-column of V makes row 64 of the O^T psum the softmax
    denominator; per q-chunk it is reciprocal'd on DVE (reciprocal_approx_fast)
    and broadcast across partitions with a K=1 ones matmul on the PE --
    no DRAM bounce.
  - scores are computed TRANSPOSED (S^T[k,q]) so exp runs on ACT out of psum
    [128,1024] (one inst per head-pair per k-chunk) and AV contracts k on
    partitions.  No max subtraction (scores ~N(0,1) after /8 scaling).
"""

import os

import numpy as np

import concourse.bass as bass
import concourse.tile as tile
from concourse import bacc, mybir
from concourse import bass_utils

F32 = mybir.dt.float32
F32R = mybir.dt.float32r
BF16 = mybir.dt.bfloat16

EMBED = 1024
SEQ = 2048
NB = 2  # batch
HEADS = 16
D = 64  # head dim
NCORES = 8
GROUPS = 4  # head groups (tensor parallel)
HG = HEADS // GROUPS  # heads per core = 4
DG = HG * D  # dims per core = 256
KC = EMBED // 128  # 8 contraction chunks for projections
TCH = 512  # token chunk (projection granularity == q-chunk granularity)
NT = SEQ // TCH  # 4 chunks
QC = 512  # q tokens per attention block

_MM_DTYPE_NAME = "bfloat16"

# set by run_cores(); test.py reads exec_time_ns from here
LAST_RESULTS = None
_CACHED_NC = {}


def build_nc():
    nc = bacc.Bacc("TRN2", target_bir_lowering=False, debug=False,
                   num_devices=NCORES)

    xT = nc.dram_tensor("xT", (EMBED, SEQ), BF16, kind="ExternalInput").ap()
    wq = nc.dram_tensor("wq", (EMBED, DG), BF16, kind="ExternalInput").ap()
    wk = nc.dram_tensor("wk", (EMBED, DG), BF16, kind="ExternalInput").ap()
    wv = nc.dram_tensor("wv", (EMBED, DG), BF16, kind="ExternalInput").ap()
    wo = nc.dram_tensor("wo", (DG, EMBED), BF16, kind="ExternalInput").ap()
    y = nc.dram_tensor("y", (SEQ, EMBED), F32, kind="ExternalOutput").ap()

    with tile.TileContext(nc) as tc:
        with (
            tc.tile_pool(name="weights", bufs=1) as wpool,
            tc.tile_pool(name="qk", bufs=1) as qkpool,
            tc.tile_pool(name="vpool", bufs=1) as vpool,
            tc.tile_pool(name="otpool", bufs=1) as otpool,
            tc.tile_pool(name="xchunk", bufs=1) as xpool,
            tc.tile_pool(name="epool", bufs=4) as epool,
            tc.tile_pool(name="stage", bufs=2) as stpool,
            tc.tile_pool(name="rbp", bufs=4) as rbpool,
            tc.tile_pool(name="den", bufs=1) as denpool,
            tc.tile_pool(name="ystage", bufs=3) as ypool,
            tc.tile_pool(name="ps_sc", bufs=2, space="PSUM") as psB,  # scores
            tc.tile_pool(name="ps_po", bufs=1, space="PSUM") as psA,  # O^T acc
            tc.tile_pool(name="ps_mc", bufs=2, space="PSUM") as psC,  # proj/fc/rb
        ):
            # ---- persistent tiles ----
            wq_sb = wpool.tile([128, KC, DG], BF16, name="wq_sb", tag="wq")
            wk_sb = wpool.tile([128, KC, DG], BF16, name="wk_sb", tag="wk")
            wv_sb = wpool.tile([128, KC, DG], BF16, name="wv_sb", tag="wv")
            wo_sb = wpool.tile([128, DG // 128, EMBED], BF16, name="wo_sb",
                               tag="wo")
            QTs = [qkpool.tile([128, 2, TCH], BF16, name=f"qt{t}", tag=f"qt{t}")
                   for t in range(NT)]
            KTs = [qkpool.tile([128, 2, TCH], BF16, name=f"kt{t}", tag=f"kt{t}")
                   for t in range(NT)]
            Vs = [vpool.tile([128, 4, HG, D + 1], BF16, name=f"v{t}",
                             tag=f"v{t}") for t in range(NT)]
            OT2 = otpool.tile([128, 2, SEQ], BF16, name="ot2", tag="ot2")
            xcs = [xpool.tile([128, KC, TCH], BF16, name=f"xc{t}", tag=f"xc{t}")
                   for t in range(NT)]
            # ones row for the K=1 reciprocal-broadcast matmul
            ones_b = denpool.tile([1, D], BF16, name="ones_b", tag="ones")
            warm_in = denpool.tile([1, 8], F32, name="warm_in", tag="wi")
            warm_out = denpool.tile([1, 8], BF16, name="warm_out", tag="wo2")

            dm_l = denpool.tile([128, 128], BF16, name="dm_l", tag="dml")
            dm_r = denpool.tile([128, 512], BF16, name="dm_r", tag="dmr")

            # ---- input DMAs, split across the two HWDGE queues ----
            # x0 kc-pieces alternate between queues so the K0 projection
            # chain starts mid-transfer; weights interleave by first use.
            xTr = xT.rearrange("(c p) s -> p c s", p=128)
            nc.sync.dma_start(out=wk_sb,
                              in_=wk.rearrange("(c p) n -> p c n", p=128))
            nc.scalar.dma_start(out=wq_sb,
                                in_=wq.rearrange("(c p) n -> p c n", p=128))
            for kc in range(KC):
                q = nc.sync if kc % 2 == 0 else nc.scalar
                q.dma_start(out=xcs[0][:, kc:kc + 1, :],
                            in_=xTr[:, kc:kc + 1, 0:TCH])
            nc.scalar.dma_start(out=wv_sb,
                                in_=wv.rearrange("(c p) n -> p c n", p=128))
            for t in range(1, NT):
                q = nc.sync if t % 2 == 1 else nc.scalar
                q.dma_start(out=xcs[t],
                            in_=xTr[:, :, t * TCH:(t + 1) * TCH])
            nc.scalar.dma_start(out=wo_sb,
                                in_=wo.rearrange("(c p) n -> p c n", p=128))

            # constants: ones column 0 of V (denominator lands in po row 0,
            # base partition 0, so the reciprocal runs in place); ACT
            # exp-table warmup; zero tiles for the PE p-state warmup matmuls
            nc.vector.memset(dm_l, 0.0)
            nc.vector.memset(dm_r, 0.0)
            for t in range(NT):
                nc.vector.memset(Vs[t][:, :, :, 0:1], 1.0)
            nc.vector.memset(ones_b, 1.0)
            nc.vector.memset(warm_in, 0.0)
            nc.scalar.activation(out=warm_out, in_=warm_in,
                                 func=mybir.ActivationFunctionType.Exp,
                                 scale=1.0)

            # PE p-state warmup: the tensor engine ramps 0.65 -> 1.2 -> 2.4GHz
            # only over ~3us of gap-free execution.  Burn the DMA-prologue
            # time ramping on throwaway matmuls so projections run at speed.
            for _ in range(20):
                psd = psC.tile([128, 512], F32, name="pd", tag="pc")
                nc.tensor.matmul(psd, dm_l, dm_r, start=True, stop=True)

            # ---- projection pieces (emitted interleaved with attention) ----
            def emit_qk(wsb, dst, t, mt):
                ps = psC.tile([128, 512], F32, name="pp", tag="pc")
                for kc in range(KC):
                    nc.tensor.matmul(
                        ps,
                        wsb[:, kc, mt * 128:(mt + 1) * 128],
                        xcs[t][:, kc, :],
                        start=(kc == 0),
                        stop=(kc == KC - 1),
                    )
                nc.vector.tensor_copy(out=dst[t][:, mt, :], in_=ps)

            def emit_v(t, ti):
                ps = psC.tile([128, 512], F32, name="pv", tag="pc")
                for kc in range(KC):
                    nc.tensor.matmul(
                        ps[:, 0:DG],
                        xcs[t][:, kc, ti * 128:(ti + 1) * 128],
                        wv_sb[:, kc, :],
                        start=(kc == 0),
                        stop=(kc == KC - 1),
                    )
                nc.vector.tensor_copy(
                    out=Vs[t][:, ti, :, 1:D + 1],
                    in_=ps[:, 0:DG].rearrange("p (h d) -> p h d", h=HG))

            rdens = {}  # (qcb, hm) -> [rden_b tiles j=0,1], set at drain time

            def emit_rb_norm(qcb, hm):
                # broadcast each head's 1/den row onto its 64 dim-partitions
                # (two K=1 ones-matmuls), then normalize O^T in place
                qs = slice(qcb * QC, (qcb + 1) * QC)
                rb = psC.tile([128, QC], F32, name="rb", tag="pc")
                for j in range(2):
                    nc.tensor.matmul(rb[j * D:(j + 1) * D, :], ones_b,
                                     rdens[(qcb, hm)][j],
                                     start=True, stop=True)
                nc.vector.tensor_mul(OT2[:, hm, qs], OT2[:, hm, qs], rb)

            def emit_fc(qcb, k):
                # one (token-block, out-column-block) piece of the partial
                # fc_out for q-chunk qcb
                tt, nch = divmod(k, EMBED // 512)
                trow = qcb * QC + tt * 128
                ps = psC.tile([128, 512], F32, name="fo", tag="pc")
                for hm in range(2):
                    nc.tensor.matmul(
                        ps,
                        OT2[:, hm, trow:trow + 128],
                        wo_sb[:, hm, nch * 512:(nch + 1) * 512],
                        start=(hm == 0),
                        stop=(hm == 1),
                    )
                ys = ypool.tile([128, 512], F32, name="ys", tag="ys")
                nc.vector.tensor_copy(out=ys, in_=ps)
                nc.sync.dma_start(
                    out=y[trow:trow + 128, nch * 512:(nch + 1) * 512],
                    in_=ys)

            def emit_piece(p):
                kind = p[0]
                if kind == "K":
                    emit_qk(wk_sb, KTs, p[1], p[2])
                elif kind == "Q":
                    emit_qk(wq_sb, QTs, p[1], p[2])
                elif kind == "V":
                    emit_v(p[1], p[2])
                elif kind == "RB":
                    emit_rb_norm(p[1], p[2])
                elif kind == "FC":
                    emit_fc(p[1], p[2])

            # minimal prologue: just enough for (qc0, hm0) scores + first AV.
            # K0/Q0 mt0 run as interleaved kc-chains so both consume the
            # streaming x0 pieces as they land instead of serializing.
            psk = psC.tile([128, 512], F32, name="ppk", tag="pc")
            psq = psC.tile([128, 512], F32, name="ppq", tag="pc")
            for kc in range(KC):
                nc.tensor.matmul(psk, wk_sb[:, kc, 0:128], xcs[0][:, kc, :],
                                 start=(kc == 0), stop=(kc == KC - 1))
                nc.tensor.matmul(psq, wq_sb[:, kc, 0:128], xcs[0][:, kc, :],
                                 start=(kc == 0), stop=(kc == KC - 1))
            nc.vector.tensor_copy(out=KTs[0][:, 0, :], in_=psk)
            nc.vector.tensor_copy(out=QTs[0][:, 0, :], in_=psq)
            for p in (("K", 0, 1), ("V", 0, 0)):
                emit_piece(p)

            # remaining projections just-in-time inside (qc0, hm) m-loops
            # (chunk t's K before scores m=4t, V_ti before AV m=4t+ti);
            # q-chunk qcb's epilogue (normalize-broadcast RB, fc_out FC) is
            # deferred into qcb+1's m-loops so the PE queue never head-of-line
            # blocks on the DVE/DMA epilogue chain.
            schedule = {
                (0, 0): {
                    0: [("V", 0, 1)],
                    1: [("V", 0, 2), ("K", 1, 0)],
                    2: [("V", 0, 3), ("K", 1, 1)],
                    3: [("V", 1, 0)],
                    4: [("V", 1, 1), ("K", 2, 0)],
                    5: [("V", 1, 2), ("K", 2, 1)],
                    6: [("V", 1, 3)],
                    7: [("V", 2, 0), ("K", 3, 0)],
                    8: [("V", 2, 1), ("K", 3, 1)],
                    9: [("V", 2, 2)], 10: [("V", 2, 3)],
                    11: [("V", 3, 0)], 12: [("V", 3, 1)],
                    13: [("V", 3, 2)], 14: [("V", 3, 3)],
                    15: [("Q", 0, 1)],
                },
                (0, 1): {
                    1: [("Q", 1, 0)], 3: [("Q", 1, 1)],
                },
                (1, 0): {
                    1: [("RB", 0, 0)],
                    3: [("RB", 0, 1), ("Q", 2, 0)],
                    4: [("FC", 0, 0)], 5: [("FC", 0, 1)],
                    6: [("Q", 2, 1)],
                    7: [("FC", 0, 2)], 8: [("FC", 0, 3)],
                },
                (1, 1): {
                    1: [("Q", 3, 0)], 2: [("FC", 0, 4)],
                    3: [("Q", 3, 1)], 4: [("FC", 0, 5)],
                    5: [("FC", 0, 6)], 6: [("FC", 0, 7)],
                },
                (2, 0): {
                    1: [("RB", 1, 0)], 3: [("RB", 1, 1)],
                    4: [("FC", 1, 0)], 5: [("FC", 1, 1)],
                    7: [("FC", 1, 2)], 8: [("FC", 1, 3)],
                },
                (2, 1): {
                    2: [("FC", 1, 4)], 3: [("FC", 1, 5)],
                    5: [("FC", 1, 6)], 6: [("FC", 1, 7)],
                },
                (3, 0): {
                    1: [("RB", 2, 0)], 3: [("RB", 2, 1)],
                    4: [("FC", 2, 0)], 5: [("FC", 2, 1)],
                    7: [("FC", 2, 2)], 8: [("FC", 2, 3)],
                },
                (3, 1): {
                    2: [("FC", 2, 4)], 3: [("FC", 2, 5)],
                    5: [("FC", 2, 6)], 6: [("FC", 2, 7)],
                    # qc3/hm0's normalize can run during this m-loop; only
                    # hm1's epilogue + FC(3) remain for the tail
                    8: [("RB", 3, 0)],
                },
            }

            # ---- attention + per-q-chunk epilogue ----
            for qcb in range(SEQ // QC):
                qs = slice(qcb * QC, (qcb + 1) * QC)
                for hm in range(2):
                    po = [psA.tile([D + 1, QC], F32, name=f"po{j}",
                                   tag=f"po{j}") for j in range(2)]
                    for m in range(SEQ // 128):
                        for p in schedule.get((qcb, hm), {}).get(m, []):
                            emit_piece(p)
                        ps = psB.tile([128, 2 * QC], F32, name="sc", tag="sc")
                        for j in range(2):
                            nc.tensor.matmul(
                                ps[:, j * QC:(j + 1) * QC],
                                KTs[m // 4][j * D:(j + 1) * D, hm,
                                            (m % 4) * 128:(m % 4 + 1) * 128],
                                QTs[qcb][j * D:(j + 1) * D, hm, :],
                                start=True,
                                stop=True,
                            )
                        e = epool.tile([128, 2 * QC], BF16, name="e", tag="e")
                        nc.scalar.activation(
                            out=e, in_=ps,
                            func=mybir.ActivationFunctionType.Exp,
                            scale=1.0 / np.sqrt(D),
                        )
                        for j in range(2):
                            nc.tensor.matmul(
                                po[j],
                                Vs[m // 4][:, m % 4, 2 * hm + j, :],
                                e[:, j * QC:(j + 1) * QC],
                                start=(m == 0),
                                stop=(m == SEQ // 128 - 1),
                            )
                    # drain O^T (row 0 = denominator) for this (q-chunk,
                    # pair); the st copies free the po psum banks, then the
                    # denominator is reciprocal'd in place and staged to bf16
                    # for the RB broadcast matmul
                    sts = []
                    for j in range(2):
                        st = stpool.tile([D + 1, QC], F32, name="st", tag="st")
                        nc.vector.tensor_copy(out=st, in_=po[j])
                        sts.append(st)
                    rpair = []
                    for j, st in enumerate(sts):
                        # casting DMA (f32 -> bf16, partition remap) via
                        # software DGE on gpsimd
                        nc.gpsimd.dma_start(
                            out=OT2[j * D:(j + 1) * D, hm, qs],
                            in_=st[1:D + 1, :])
                        nc.vector.reciprocal_approx_fast(
                            out=st[0:1, :], in_=st[0:1, :])
                        rden_b = rbpool.tile([1, QC], BF16, name="rden_b",
                                             tag="rden")
                        nc.vector.tensor_copy(out=rden_b, in_=st[0:1, :])
                        rpair.append(rden_b)
                    rdens[(qcb, hm)] = rpair

            # tail: last q-chunk's hm1 epilogue + its fc_out
            emit_rb_norm(NT - 1, 1)
            for k in range(8):
                emit_fc(NT - 1, k)

    nc.compile()
    return nc


def shard_inputs(x, Wv, Wk, Wq, Wo):
    """Build the 8 per-core input maps."""
    import ml_dtypes
    wire = ml_dtypes.bfloat16
    in_maps = []
    for c in range(NCORES):
        n, g = divmod(c, GROUPS)
        cols = slice(g * DG, (g + 1) * DG)
        in_maps.append({
            "xT": np.ascontiguousarray(
                np.asarray(x[n], np.float32).T).astype(wire),
            "wq": np.ascontiguousarray(
                np.asarray(Wq, np.float32)[:, cols]).astype(wire),
            "wk": np.ascontiguousarray(
                np.asarray(Wk, np.float32)[:, cols]).astype(wire),
            "wv": np.ascontiguousarray(
                np.asarray(Wv, np.float32)[:, cols]).astype(wire),
            "wo": np.ascontiguousarray(
                np.asarray(Wo, np.float32)[cols, :]).astype(wire),
        })
    return in_maps


def kernel(x, Wv, Wk, Wq, Wo, bo):
    global LAST_RESULTS
    x = np.asarray(x, np.float32)
    in_maps = shard_inputs(x, Wv, Wk, Wq, Wo)

    if "nc" not in _CACHED_NC:
        _CACHED_NC["nc"] = build_nc()
    nc = _CACHED_NC["nc"]

    trace = os.environ.get("MHA_TRACE", "0") == "1"
    res = bass_utils.run_bass_kernel_spmd(
        nc, in_maps, core_ids=list(range(NCORES)), trace=trace)
    LAST_RESULTS = res

    bo = np.asarray(bo, np.float32)
    out = np.empty((NB, SEQ, EMBED), np.float32)
    for n in range(NB):
        acc = res.results[n * GROUPS]["y"].astype(np.float32).copy()
        for g in range(1, GROUPS):
            acc += res.results[n * GROUPS + g]["y"]
        out[n] = acc + bo[None, :]
    return out
